# revision 38
# baseline (speedup 1.0000x reference)
"""EntityAggrNet (2-layer GNN message passing) on 8 Trainium2 NeuronCores.

Strategy (v5)
-------------
Node-parallel sharding: core w owns nodes [w*2048, (w+1)*2048).  Edges are
sorted by src on the host; each core processes the edges whose src lands in
its node range.

Per layer, per core:
  1. Per 128-src-node window the host DEDUPS the dst set; each distinct row
     is dma_gathered once (fp8, 256 B rows) from a replicated HBM copy of
     the layer input.  Calls are <=1024 idxs (the single_packet 64-descs/
     engine packet cap) and striped across all 4 SWDGE queues, which
     quadruples Q7 descriptor-generation throughput -- SWDGE descgen
     (~9 ns/desc/queue) is the binding resource of the whole kernel.
  2. Segment-sum via selector matmuls: multi-hot COUNT selectors uploaded
     from the host once (fp8, resident, reused by both layers) route each
     gathered row to every src slot referencing it, two chunks at a time
     with fp8 DoubleRow matmuls.  Mean = PSUM * (1/cnt) on evacuation
     (scalar engine).
  3. Linearity moves the weight matmuls out of the edge loop:
     mean(x[dst]) @ W_msg.  The edge-feature path collapses to
     Hn @ (emb_table @ W_edge) where Hn[n, d] = count(src=n, feat=d)/cnt[n]
     is a host-computed *index* matrix; biases ride along as extra Hn rows.
  4. Dense phase in feature-major layout (PE transposes), BatchNorm stats
     via a 2KB AllReduce; BN+ReLU applied feature-major on the scalar
     engine, then PE-transposed back to node-major for shipping.
  5. Layer output is AllGathered in fp8 (4 MB, ~24 us RDH) to become the
     next layer's gather source.

Scheduling notes: both HWDGE queues are in-order FIFOs -- the selector
loads ride the scalar queue ahead of anything collective-dependent, and
the warmup AllReduce (absorbs the ~75 us ncfw cold start) reduces garbage
so nothing waits on it.  Measured ~474 us HW exec, rel err ~1.1e-2
(fp8e4 edge quantization; dense path stays float32r).
"""
import os
import sys

if "/opt/trn_rl_repo" not in sys.path:
    sys.path.insert(0, "/opt/trn_rl_repo")

import numpy as np

import concourse.bass as bass  # noqa: F401  (engine types referenced via nc)
import concourse.tile as tile
from concourse import bacc, mybir
from concourse import bass_utils
from concourse.bass_interp import get_hw_module

F32 = mybir.dt.float32
F32R = mybir.dt.float32r
I16 = mybir.dt.int16
BF16 = mybir.dt.bfloat16
FP8 = mybir.dt.float8e4
ALU = mybir.AluOpType
ACTF = mybir.ActivationFunctionType
DROW = mybir.MatmulPerfMode.DoubleRow

EPS = 1e-5
B, S, D = 32, 512, 256
N = B * S                # 16384 nodes
DS, DD = 64, 64          # embedding table: [DS, DD]
L = 2                    # layers
NCORE = 8
NPC = N // NCORE         # 2048 nodes per core
WIN = 128                # nodes per PSUM window
NWIN = NPC // WIN        # 16 windows per core
NWING = N // WIN         # 128 windows globally

_CACHE = {}


def _build(nch):
    """Build + schedule + bacc-compile the SPMD program.

    nch: chunks (of 128 edges) per 128-node window, even, uniform across
    cores (host pads every window to nch*128 edges).
    """
    assert nch % 2 == 0
    cap = nch * WIN                  # padded edges per window
    epc = NWIN * cap                 # padded edges per core

    nc = bacc.Bacc("TRN2", target_bir_lowering=False, debug=False,
                   num_devices=NCORE, num_swdge_queues=4)

    # ---- I/O ----
    x0 = nc.dram_tensor("x0", [N, D], FP8, kind="ExternalInput")
    xT0 = nc.dram_tensor("xT0", [D, NPC], F32R, kind="ExternalInput")
    idx_in = nc.dram_tensor("idx_in", [128, epc // 16], I16, kind="ExternalInput")
    sel_in = nc.dram_tensor("sel_in", [128, NWIN * cap], FP8, kind="ExternalInput")
    recip_in = nc.dram_tensor("recip_in", [128, NWIN], F32, kind="ExternalInput")
    hnt_in = nc.dram_tensor("hnt_in", [128, NPC], BF16, kind="ExternalInput")
    ident_in = nc.dram_tensor("ident_in", [128, 128], F32, kind="ExternalInput")
    identr_in = nc.dram_tensor("identr_in", [128, 128], F32R, kind="ExternalInput")
    wm_in = [nc.dram_tensor(f"wm{l}", [D, D], F32R, kind="ExternalInput") for l in range(L)]
    ws_in = [nc.dram_tensor(f"ws{l}", [D, D], F32R, kind="ExternalInput") for l in range(L)]
    embT_in = nc.dram_tensor("embT", [DD, DS], F32R, kind="ExternalInput")
    we_in = [nc.dram_tensor(f"we{l}", [DD, D], F32R, kind="ExternalInput") for l in range(L)]
    # rows DD..127 of the EW lhsT: [bm, be, bs, zeros...] packed on host
    ewc_in = [nc.dram_tensor(f"ewc{l}", [128 - DD, D], BF16, kind="ExternalInput")
              for l in range(L)]
    gam_in = [nc.dram_tensor(f"gam{l}", [D, 1], F32, kind="ExternalInput") for l in range(L)]
    bet_in = [nc.dram_tensor(f"bet{l}", [D, 1], F32, kind="ExternalInput") for l in range(L)]
    out_ext = nc.dram_tensor("out", [NPC, D], F32, kind="ExternalOutput")

    with tile.TileContext(nc) as tc:
        with tc.tile_pool(name="const", bufs=1) as cp, \
             tc.tile_pool(name="gat", bufs=4) as gp, \
             tc.tile_pool(name="xmaj", bufs=1) as xp, \
             tc.tile_pool(name="psE", bufs=4, space="PSUM") as psE, \
             tc.tile_pool(name="psT", bufs=2, space="PSUM") as psT, \
             tc.tile_pool(name="psD", bufs=2, space="PSUM") as psD, \
             tc.tile_pool(name="dram", bufs=1, space="DRAM") as dp:

            # ---- constants into SBUF ----
            idx_all = cp.tile([128, epc // 16], I16)
            sel = cp.tile([128, NWIN, nch // 2, 2, 128], FP8)
            recip = cp.tile([128, NWIN], F32)
            hnt = cp.tile([128, NPC], BF16)
            ident = cp.tile([128, 128], F32)
            identr = cp.tile([128, 128], F32R)
            nc.sync.dma_start(out=idx_all[:, 0:cap // 16], in_=idx_in[:, 0:cap // 16])
            nc.sync.dma_start(out=idx_all[:, cap // 16:], in_=idx_in[:, cap // 16:])
            # selector rides the scalar HWDGE queue (parallel to the sync
            # queue's big loads), early windows first so matmuls start early.
            # NOTHING that waits on a collective may precede these loads on
            # the scalar queue (in-order FIFO).
            sel_flat = sel[:, :, :, :, :].rearrange("p w c t j -> p (w c t j)")
            for a, b in ((0, cap), (cap, 2 * cap), (2 * cap, 4 * cap),
                         (4 * cap, 8 * cap), (8 * cap, NWIN * cap)):
                nc.scalar.dma_start(out=sel_flat[:, a:b], in_=sel_in[:, a:b])

            nc.sync.dma_start(out=recip[:, :], in_=recip_in[:, :])
            nc.sync.dma_start(out=hnt[:, :], in_=hnt_in[:, :])
            nc.sync.dma_start(out=ident[:, :], in_=ident_in[:, :])
            nc.sync.dma_start(out=identr[:, :], in_=identr_in[:, :])

            wm_sb, ws_sb, ew_sb = [], [], []
            embT_sb = cp.tile([DD, DS], F32R)
            nc.sync.dma_start(out=embT_sb[:, :], in_=embT_in[:, :])
            for l in range(L):
                wm = cp.tile([128, 2, D], F32R, name=f"wm_sb{l}")
                ws = cp.tile([128, 2, D], F32R, name=f"ws_sb{l}")
                for kt in range(2):
                    nc.sync.dma_start(out=wm[:, kt, :], in_=wm_in[l][kt * 128:(kt + 1) * 128, :])
                    nc.sync.dma_start(out=ws[:, kt, :], in_=ws_in[l][kt * 128:(kt + 1) * 128, :])
                wm_sb.append(wm)
                ws_sb.append(ws)

                we = cp.tile([DD, D], F32R, name=f"we_sb{l}")
                nc.sync.dma_start(out=we[:, :], in_=we_in[l][:, :])
                psew = psT.tile([DD, D], F32, tag="pst", name=f"psew{l}")
                nc.tensor.matmul(psew[:, :], embT_sb[:, :], we[:, :], start=True, stop=True)
                ew = cp.tile([128, D], BF16, name=f"ew_sb{l}")
                nc.vector.tensor_copy(ew[0:DD, :], psew[:, :])
                nc.sync.dma_start(out=ew[DD:128, :], in_=ewc_in[l][:, :])
                ew_sb.append(ew)

            gb_sb = []  # [128, 2] gamma / beta per layer, packed per feat-half
            for l in range(L):
                gam = cp.tile([128, 2], F32, name=f"gam_sb{l}")
                bet = cp.tile([128, 2], F32, name=f"bet_sb{l}")
                for f in range(2):
                    nc.sync.dma_start(out=gam[:, f:f + 1], in_=gam_in[l][f * 128:(f + 1) * 128, :])
                    nc.sync.dma_start(out=bet[:, f:f + 1], in_=bet_in[l][f * 128:(f + 1) * 128, :])
                gb_sb.append((gam, bet))

            xT_cur = [cp.tile([128, NPC], F32R, name=f"xT0_sb{f}") for f in range(2)]
            for f in range(2):
                nc.sync.dma_start(out=xT_cur[f][:, :], in_=xT0[f * 128:(f + 1) * 128, :])

            # warmup collective: absorbs the ~75us ncfw cold start under the
            # L0 edge phase. Reduces garbage (the value is never read) so it
            # has no data dependencies and triggers immediately; the in-order
            # cc stream serializes it before the L0 stats AllReduce. Nothing
            # waits on its completion.
            warm_in = dp.tile([128, 1], F32, name="warm_in")
            warm_out = dp.tile([128, 1], F32, addr_space="Shared", name="warm_out")
            nc.gpsimd.collective_compute(
                "AllReduce", ALU.add,
                replica_groups=[list(range(NCORE))],
                ins=[warm_in[:, :]], outs=[warm_out[:, :]])

            xsrc = x0  # gather source (DRAM AP-able handle)

            qctr = [0]

            for l in range(L):
                # ================= edge phase =================
                gats = [None] * NWIN

                def ensure_gather(w, l=l, gats=gats):
                    if gats[w] is not None:
                        return gats[w]
                    g = gp.tile([128, nch, D], FP8, tag="g", name=f"g{l}_{w}")
                    # single-packet coalescing caps at 64 descs/engine (1024
                    # idxs per call); split the window into pieces striped
                    # over the 4 SWDGE queues so descgen runs 4-wide.
                    for off in range(0, cap, 1024):
                        sz = min(1024, cap - off)
                        co = w * (cap // 16) + off // 16
                        nc.gpsimd.dma_gather(
                            out_ap=g[:, off // 128:(off + sz) // 128, :],
                            in_ap=xsrc[:, :],
                            idxs_ap=idx_all[:, co:co + sz // 16],
                            num_idxs=sz, num_idxs_reg=sz,
                            elem_size=D, single_packet=True,
                            queue_num=qctr[0] % 4)
                        qctr[0] += 1
                    gats[w] = g
                    return g

                # Fused edge + dense pipeline: windows stream through; after
                # every 4th window the corresponding 512-node dense block,
                # its stat partials, and its node-major transposes fire.
                preout = [xp.tile([128, NPC], F32, tag=f"pre{f}", name=f"pre{l}_{f}")
                          for f in range(2)]
                redp = cp.tile([128, 16], F32, tag="redp", bufs=2, name=f"redp{l}")
                sqscr = xp.tile([128, 512], F32, tag="sqscr", name=f"sqscr{l}")
                msxTn = [[None] * (NPC // 512) for _ in range(2)]
                for w in range(NWIN):
                    g = ensure_gather(w)
                    for pf in range(1, 4):  # prefetch 3 windows ahead
                        ensure_gather(min(w + pf, NWIN - 1))
                    ps = psE.tile([128, D], F32, tag="pse", name=f"pse{l}_{w}")
                    for c in range(nch // 2):
                        nc.tensor.matmul(ps[:, :], sel[:, w, c, :, :], g[:, 2 * c:2 * c + 2, :],
                                         start=(c == 0), stop=(c == nch // 2 - 1),
                                         perf_mode=DROW)
                    # mean on evacuation (scalar engine; recip is per-partition)
                    msxw = xp.tile([128, D], F32, tag="msx", bufs=4,
                                   name=f"msx{l}_{w}")
                    nc.scalar.activation(msxw[:, :], ps[:, :], ACTF.Copy,
                                         bias=0.0, scale=recip[:, w:w + 1])
                    nb, wi = w // 4, w % 4
                    for f in range(2):
                        if wi == 0:
                            msxTn[f][nb] = xp.tile([128, 512], F32R, tag=f"msxT{f}",
                                                   bufs=2, name=f"msxT{l}_{f}_{nb}")
                        pt = psT.tile([128, 128], F32, tag="pst", name=f"ptm{l}_{w}_{f}")
                        nc.tensor.transpose(pt[:, :], msxw[:, f * 128:(f + 1) * 128],
                                            ident[:, :])
                        nc.vector.tensor_copy(msxTn[f][nb][:, wi * 128:(wi + 1) * 128],
                                              pt[:, :])
                    if wi != 3:
                        continue
                    # dense block for this group of 4 windows
                    cols = slice(nb * 512, (nb + 1) * 512)
                    for f in range(2):
                        pd = psD.tile([128, 512], F32, tag="psd", name=f"pd{l}_{f}_{nb}")
                        fo = slice(f * 128, (f + 1) * 128)
                        nc.tensor.matmul(pd[:, :], wm_sb[l][:, 0, fo], msxTn[0][nb][:, :],
                                         start=True, stop=False)
                        nc.tensor.matmul(pd[:, :], wm_sb[l][:, 1, fo], msxTn[1][nb][:, :],
                                         start=False, stop=False)
                        nc.tensor.matmul(pd[:, :], ws_sb[l][:, 0, fo], xT_cur[0][:, cols],
                                         start=False, stop=False)
                        nc.tensor.matmul(pd[:, :], ws_sb[l][:, 1, fo], xT_cur[1][:, cols],
                                         start=False, stop=False)
                        nc.tensor.matmul(pd[:, :], ew_sb[l][:, fo], hnt[:, cols],
                                         start=False, stop=True)
                        # evacuate + per-block column sums
                        nc.vector.tensor_scalar(preout[f][:, cols], pd[:, :],
                                                1.0, 0.0, ALU.mult, ALU.add,
                                                accum_out=redp[:, f * 4 + nb:f * 4 + nb + 1])
                        # per-block sum of squares on the scalar engine
                        nc.scalar.activation(sqscr[:, :], preout[f][:, cols],
                                             ACTF.Square, bias=0.0, scale=1.0,
                                             accum_out=redp[:, 8 + f * 4 + nb:
                                                            9 + f * 4 + nb])

                # ================= batchnorm stats =================
                red = cp.tile([128, 4], F32, tag="red", bufs=2, name=f"red{l}")
                for f in range(2):
                    nc.vector.tensor_reduce(red[:, f:f + 1], redp[:, f * 4:(f + 1) * 4],
                                            mybir.AxisListType.X, ALU.add)
                    nc.vector.tensor_reduce(red[:, 2 + f:3 + f],
                                            redp[:, 8 + f * 4:8 + (f + 1) * 4],
                                            mybir.AxisListType.X, ALU.add)

                st_in = dp.tile([128, 4], F32, name=f"st_in{l}")
                st_out = dp.tile([128, 4], F32, addr_space="Shared", name=f"st_out{l}")
                nc.scalar.dma_start(out=st_in[:, :], in_=red[:, :])
                nc.gpsimd.collective_compute(
                    "AllReduce", ALU.add,
                    replica_groups=[list(range(NCORE))],
                    ins=[st_in[:, :]], outs=[st_out[:, :]])
                red2 = cp.tile([128, 4], F32, tag="red2", bufs=2, name=f"red2{l}")
                nc.sync.dma_start(out=red2[:, :], in_=st_out[:, :])

                # mu/var -> scale/shift  (all [128, 2], feature-partition form)
                mo = cp.tile([128, 12], F32, tag="mo", bufs=2, name=f"mo{l}")
                mu, ex2, var, vare, sd, rsq = (mo[:, 0:2], mo[:, 2:4], mo[:, 4:6],
                                               mo[:, 6:8], mo[:, 8:10], mo[:, 10:12])
                nc.vector.tensor_scalar(mu, red2[:, 0:2], 1.0 / N, None, ALU.mult)
                nc.vector.tensor_scalar(ex2, red2[:, 2:4], 1.0 / N, None, ALU.mult)
                nc.vector.tensor_tensor(var, mu, mu, ALU.mult)
                nc.vector.tensor_tensor(var, ex2, var, ALU.subtract)
                nc.vector.tensor_scalar(vare, var, EPS, None, ALU.add)
                nc.scalar.activation(sd, vare, ACTF.Sqrt, bias=0.0, scale=1.0)
                nc.vector.reciprocal(rsq, sd)
                gam, bet = gb_sb[l]
                sc = cp.tile([128, 4], F32, tag="sc", bufs=2, name=f"sc{l}")
                scale2, shift2 = sc[:, 0:2], sc[:, 2:4]
                nc.vector.tensor_tensor(scale2, gam[:, :], rsq, ALU.mult)
                nc.vector.tensor_tensor(shift2, mu, scale2, ALU.mult)
                nc.vector.tensor_tensor(shift2, bet[:, :], shift2, ALU.subtract)

                # ===== BN + ReLU feature-major, transpose to node-major, ship
                last = l == L - 1
                xnT = [xp.tile([128, NPC], F32R, tag=f"xnT{f}", name=f"xnT{l}_{f}")
                       for f in range(2)]
                if not last:
                    agi = dp.tile([NPC, D], FP8, name=f"agi{l}")
                    ago = dp.tile([N, D], FP8, addr_space="Shared", name=f"ago{l}")
                xrow = xp.tile([128, NWIN, D], F32 if last else FP8,
                               tag="xrow", name=f"xrow{l}")
                dst = out_ext if last else agi
                dst_ap = dst[:, :].rearrange("(w p) d -> p w d", p=128)
                for nb in range(NWIN // 4):
                    cols = slice(nb * 512, (nb + 1) * 512)
                    for f in range(2):
                        nc.scalar.activation(xnT[f][:, cols], preout[f][:, cols],
                                             ACTF.Relu, bias=shift2[:, f:f + 1],
                                             scale=scale2[:, f:f + 1])
                    for w in range(nb * 4, nb * 4 + 4):
                        for f in range(2):
                            pt = psT.tile([128, 128], F32R, tag="pst",
                                          name=f"ptx{l}_{w}_{f}")
                            nc.tensor.transpose(pt[:, :],
                                                xnT[f][:, w * 128:(w + 1) * 128],
                                                identr[:, :])
                            nc.vector.tensor_copy(xrow[:, w, f * 128:(f + 1) * 128],
                                                  pt[:, :])
                    ws_ = slice(nb * 4, nb * 4 + 4)
                    nc.sync.dma_start(out=dst_ap[:, ws_, :], in_=xrow[:, ws_, :])

                if not last:
                    nc.gpsimd.collective_compute(
                        "AllGather", ALU.bypass,
                        replica_groups=[list(range(NCORE))],
                        ins=[agi[:, :]], outs=[ago[:, :]])
                    xsrc = ago
                    xT_cur = xnT

    nc.compile()
    nc.m = get_hw_module(nc.m)
    return nc


def _preprocess(data, edge, edge_feature):
    """Host-side index preprocessing: sort edges by src window, dedup each
    window's dst set (each distinct row is gathered once; the multi-hot
    selector routes it to every src slot that references it, with count
    weights), window-pad, build count matrices."""
    import ml_dtypes
    src = np.asarray(edge[0], dtype=np.int64)
    dst = np.asarray(edge[1], dtype=np.int64)
    ef = np.asarray(edge_feature, dtype=np.int64)

    wid_e = src // WIN
    order = np.lexsort((src, dst, wid_e))
    src_s = src[order]
    dst_s = dst[order]

    cnt = np.bincount(src, minlength=N)
    recip = (1.0 / np.maximum(cnt, 1)).astype(np.float32)
    H = np.bincount(src * DS + ef, minlength=N * DS).reshape(N, DS)
    Hn = (H * recip[:, None]).astype(np.float32)

    wid = src_s // WIN
    wcnt = np.bincount(wid, minlength=NWING)
    wstart = np.zeros(NWING + 1, np.int64)
    np.cumsum(wcnt, out=wstart[1:])

    uniqs, invs = [], []
    for g in range(NWING):
        u, inv = np.unique(dst_s[wstart[g]:wstart[g + 1]], return_inverse=True)
        uniqs.append(u)
        invs.append(inv)
    nch = max(int(np.ceil(max(len(u) for u in uniqs) / 128)), 1)
    nch += nch % 2  # DoubleRow consumes chunk pairs
    cap = nch * WIN

    idx_pad = np.zeros((NWING, cap), np.int16)
    selw = np.zeros((NWING, cap, 128), np.float16)
    for g in range(NWING):
        a, b = wstart[g], wstart[g + 1]
        idx_pad[g, :len(uniqs[g])] = uniqs[g].astype(np.int16)
        np.add.at(selw[g], (invs[g], src_s[a:b] - g * WIN), 1.0)

    per_core = []
    for w in range(NCORE):
        gsl = slice(w * NWIN, (w + 1) * NWIN)
        nsl = slice(w * NPC, (w + 1) * NPC)
        flat_idx = idx_pad[gsl].reshape(-1)           # [NWIN*cap]
        idx_tile = np.tile(flat_idx.reshape(-1, 16).T, (8, 1)).astype(np.int16)
        # selector: [p, (w c j)] with p = row slot in chunk, j = src slot
        selc = selw[gsl].reshape(NWIN, nch, 128, 128)  # [w, c, p, j]
        sel_tile = np.ascontiguousarray(
            selc.transpose(2, 0, 1, 3).reshape(128, NWIN * cap)
        ).astype(ml_dtypes.float8_e4m3)
        recip_sw = recip[nsl].reshape(NWIN, 128).T.copy()    # [128, NWIN]
        hnt = np.zeros((128, NPC), np.float32)
        hnt[:DS, :] = Hn[nsl].T
        nz = (cnt[nsl] > 0).astype(np.float32)
        hnt[DS, :] = nz
        hnt[DS + 1, :] = nz
        hnt[DS + 2, :] = 1.0
        hnt = hnt.astype(ml_dtypes.bfloat16)
        xT0 = np.ascontiguousarray(
            data.reshape(N, D)[nsl].T.astype(np.float32))
        per_core.append(dict(idx_in=idx_tile, sel_in=sel_tile,
                             recip_in=recip_sw, hnt_in=hnt, xT0=xT0))
    return nch, per_core


def kernel(data, emb_table, W_msg, b_msg, W_self, b_self, W_edge, b_edge,
           bn_gamma, bn_beta, edge, edge_feature):
    data = np.asarray(data)
    nch, per_core = _preprocess(data, np.asarray(edge), np.asarray(edge_feature))

    if nch not in _CACHE:
        _CACHE[nch] = _build(nch)
    nc = _CACHE[nch]

    import ml_dtypes
    x0 = np.ascontiguousarray(data.reshape(N, D).astype(ml_dtypes.float8_e4m3))
    ident = np.eye(128, dtype=np.float32)
    common = {
        "x0": x0, "ident_in": ident, "identr_in": ident,
        "embT": np.ascontiguousarray(np.asarray(emb_table, np.float32).T),
    }
    for l in range(L):
        common[f"wm{l}"] = np.ascontiguousarray(np.asarray(W_msg[l], np.float32))
        common[f"ws{l}"] = np.ascontiguousarray(np.asarray(W_self[l], np.float32))
        common[f"we{l}"] = np.ascontiguousarray(np.asarray(W_edge[l], np.float32))
        ewc = np.zeros((128 - DD, D), np.float32)
        ewc[0] = np.asarray(b_msg[l], np.float32)
        ewc[1] = np.asarray(b_edge[l], np.float32)
        ewc[2] = np.asarray(b_self[l], np.float32)
        common[f"ewc{l}"] = ewc.astype(ml_dtypes.bfloat16)
        common[f"gam{l}"] = np.asarray(bn_gamma[l], np.float32).reshape(D, 1)
        common[f"bet{l}"] = np.asarray(bn_beta[l], np.float32).reshape(D, 1)

    in_maps = [{**common, **pc} for pc in per_core]
    trace = bool(os.environ.get("GNN_TRN_TRACE"))
    res = bass_utils.run_bass_kernel_spmd(
        nc, in_maps, core_ids=list(range(NCORE)), trace=trace)
    if trace:
        global LAST_RESULT
        LAST_RESULT = res
    out = np.concatenate([res.results[c]["out"] for c in range(NCORE)], axis=0)
    return out.reshape(B, S, D).astype(np.float32)


LAST_RESULT = None


# revision 39
# speedup vs baseline: 1.1089x; 1.1089x over previous
"""EntityAggrNet (2-layer GNN message passing) on 8 Trainium2 NeuronCores.

Strategy (v5)
-------------
Node-parallel sharding: core w owns nodes [w*2048, (w+1)*2048).  Edges are
sorted by src on the host; each core processes the edges whose src lands in
its node range.

Per layer, per core:
  1. Per 128-src-node window the host DEDUPS the dst set; each distinct row
     is dma_gathered once (fp8, 256 B rows) from a replicated HBM copy of
     the layer input.  Calls are <=1024 idxs (the single_packet 64-descs/
     engine packet cap) and striped across all 4 SWDGE queues, which
     quadruples Q7 descriptor-generation throughput -- SWDGE descgen
     (~9 ns/desc/queue) is the binding resource of the whole kernel.
  2. Segment-sum via selector matmuls: multi-hot COUNT selectors uploaded
     from the host once (fp8, resident, reused by both layers) route each
     gathered row to every src slot referencing it, two chunks at a time
     with fp8 DoubleRow matmuls.  Mean = PSUM * (1/cnt) on evacuation
     (scalar engine).
  3. Linearity moves the weight matmuls out of the edge loop:
     mean(x[dst]) @ W_msg.  The edge-feature path collapses to
     Hn @ (emb_table @ W_edge) where Hn[n, d] = count(src=n, feat=d)/cnt[n]
     is a host-computed *index* matrix; biases ride along as extra Hn rows.
  4. Dense phase in feature-major layout (PE transposes), BatchNorm stats
     via a 2KB AllReduce; BN+ReLU applied feature-major on the scalar
     engine, then PE-transposed back to node-major for shipping.
  5. Layer output is AllGathered in fp8 (4 MB, ~24 us RDH) to become the
     next layer's gather source.

Scheduling notes: both HWDGE queues are in-order FIFOs -- the selector
loads ride the scalar queue ahead of anything collective-dependent, and
the warmup AllReduce (absorbs the ~75 us ncfw cold start) reduces garbage
so nothing waits on it.  Measured ~474 us HW exec, rel err ~1.1e-2
(fp8e4 edge quantization; dense path stays float32r).
"""
import os
import sys

if "/opt/trn_rl_repo" not in sys.path:
    sys.path.insert(0, "/opt/trn_rl_repo")

import numpy as np

import concourse.bass as bass  # noqa: F401  (engine types referenced via nc)
import concourse.tile as tile
from concourse import bacc, mybir
from concourse import bass_utils
from concourse.bass_interp import get_hw_module

F32 = mybir.dt.float32
F32R = mybir.dt.float32r
I16 = mybir.dt.int16
BF16 = mybir.dt.bfloat16
FP8 = mybir.dt.float8e4
ALU = mybir.AluOpType
ACTF = mybir.ActivationFunctionType
DROW = mybir.MatmulPerfMode.DoubleRow

EPS = 1e-5
B, S, D = 32, 512, 256
N = B * S                # 16384 nodes
DS, DD = 64, 64          # embedding table: [DS, DD]
L = 2                    # layers
NCORE = 8
NPC = N // NCORE         # 2048 nodes per core
WIN = 128                # nodes per PSUM window
NWIN = NPC // WIN        # 16 windows per core
NWING = N // WIN         # 128 windows globally

_CACHE = {}


def _build(nch):
    """Build + schedule + bacc-compile the SPMD program.

    nch: chunks (of 128 edges) per 128-node window, even, uniform across
    cores (host pads every window to nch*128 edges).
    """
    assert nch % 2 == 0
    cap = nch * WIN                  # padded edges per window
    epc = NWIN * cap                 # padded edges per core

    nc = bacc.Bacc("TRN2", target_bir_lowering=False, debug=False,
                   num_devices=NCORE, num_swdge_queues=4)

    # ---- I/O ----
    x0 = nc.dram_tensor("x0", [N, D], FP8, kind="ExternalInput")
    xT0 = nc.dram_tensor("xT0", [D, NPC], F32R, kind="ExternalInput")
    idx_in = nc.dram_tensor("idx_in", [128, epc // 16], I16, kind="ExternalInput")
    sel_in = nc.dram_tensor("sel_in", [128, NWIN * cap], FP8, kind="ExternalInput")
    recip_in = nc.dram_tensor("recip_in", [128, NWIN], F32, kind="ExternalInput")
    hnt_in = nc.dram_tensor("hnt_in", [128, NPC], BF16, kind="ExternalInput")
    ident_in = nc.dram_tensor("ident_in", [128, 128], F32, kind="ExternalInput")
    identr_in = nc.dram_tensor("identr_in", [128, 128], F32R, kind="ExternalInput")
    wm_in = [nc.dram_tensor(f"wm{l}", [D, D], F32R, kind="ExternalInput") for l in range(L)]
    ws_in = [nc.dram_tensor(f"ws{l}", [D, D], F32R, kind="ExternalInput") for l in range(L)]
    embT_in = nc.dram_tensor("embT", [DD, DS], F32R, kind="ExternalInput")
    we_in = [nc.dram_tensor(f"we{l}", [DD, D], F32R, kind="ExternalInput") for l in range(L)]
    # rows DD..127 of the EW lhsT: [bm, be, bs, zeros...] packed on host
    ewc_in = [nc.dram_tensor(f"ewc{l}", [128 - DD, D], BF16, kind="ExternalInput")
              for l in range(L)]
    gam_in = [nc.dram_tensor(f"gam{l}", [D, 1], F32, kind="ExternalInput") for l in range(L)]
    bet_in = [nc.dram_tensor(f"bet{l}", [D, 1], F32, kind="ExternalInput") for l in range(L)]
    out_ext = nc.dram_tensor("out", [NPC, D], F32, kind="ExternalOutput")

    with tile.TileContext(nc) as tc:
        with tc.tile_pool(name="const", bufs=1) as cp, \
             tc.tile_pool(name="gat", bufs=4) as gp, \
             tc.tile_pool(name="xmaj", bufs=1) as xp, \
             tc.tile_pool(name="psE", bufs=4, space="PSUM") as psE, \
             tc.tile_pool(name="psT", bufs=2, space="PSUM") as psT, \
             tc.tile_pool(name="psD", bufs=2, space="PSUM") as psD, \
             tc.tile_pool(name="dram", bufs=1, space="DRAM") as dp:

            # ---- constants into SBUF ----
            idx_all = cp.tile([128, epc // 16], I16)
            sel = cp.tile([128, NWIN, nch // 2, 2, 128], FP8)
            recip = cp.tile([128, NWIN], F32)
            hnt = cp.tile([128, NPC], BF16)
            ident = cp.tile([128, 128], F32)
            identr = cp.tile([128, 128], F32R)
            nc.sync.dma_start(out=idx_all[:, 0:cap // 16], in_=idx_in[:, 0:cap // 16])
            nc.sync.dma_start(out=idx_all[:, cap // 16:], in_=idx_in[:, cap // 16:])
            # selector rides the scalar HWDGE queue (parallel to the sync
            # queue's big loads), early windows first so matmuls start early.
            # NOTHING that waits on a collective may precede these loads on
            # the scalar queue (in-order FIFO).
            sel_flat = sel[:, :, :, :, :].rearrange("p w c t j -> p (w c t j)")
            for a, b in ((0, cap), (cap, 4 * cap), (4 * cap, NWIN * cap)):
                nc.scalar.dma_start(out=sel_flat[:, a:b], in_=sel_in[:, a:b])

            nc.sync.dma_start(out=recip[:, :], in_=recip_in[:, :])
            nc.sync.dma_start(out=hnt[:, :], in_=hnt_in[:, :])
            nc.sync.dma_start(out=ident[:, :], in_=ident_in[:, :])
            nc.sync.dma_start(out=identr[:, :], in_=identr_in[:, :])

            wm_sb, ws_sb, ew_sb = [], [], []
            embT_sb = cp.tile([DD, DS], F32R)
            nc.sync.dma_start(out=embT_sb[:, :], in_=embT_in[:, :])
            for l in range(L):
                wm = cp.tile([128, 2, D], F32R, name=f"wm_sb{l}")
                ws = cp.tile([128, 2, D], F32R, name=f"ws_sb{l}")
                for kt in range(2):
                    nc.sync.dma_start(out=wm[:, kt, :], in_=wm_in[l][kt * 128:(kt + 1) * 128, :])
                    nc.sync.dma_start(out=ws[:, kt, :], in_=ws_in[l][kt * 128:(kt + 1) * 128, :])
                wm_sb.append(wm)
                ws_sb.append(ws)

                we = cp.tile([DD, D], F32R, name=f"we_sb{l}")
                nc.sync.dma_start(out=we[:, :], in_=we_in[l][:, :])
                psew = psT.tile([DD, D], F32, tag="pst", name=f"psew{l}")
                nc.tensor.matmul(psew[:, :], embT_sb[:, :], we[:, :], start=True, stop=True)
                ew = cp.tile([128, D], BF16, name=f"ew_sb{l}")
                nc.vector.tensor_copy(ew[0:DD, :], psew[:, :])
                nc.sync.dma_start(out=ew[DD:128, :], in_=ewc_in[l][:, :])
                ew_sb.append(ew)

            gb_sb = []  # [128, 2] gamma / beta per layer, packed per feat-half
            for l in range(L):
                gam = cp.tile([128, 2], F32, name=f"gam_sb{l}")
                bet = cp.tile([128, 2], F32, name=f"bet_sb{l}")
                for f in range(2):
                    nc.sync.dma_start(out=gam[:, f:f + 1], in_=gam_in[l][f * 128:(f + 1) * 128, :])
                    nc.sync.dma_start(out=bet[:, f:f + 1], in_=bet_in[l][f * 128:(f + 1) * 128, :])
                gb_sb.append((gam, bet))

            xT_cur = [cp.tile([128, NPC], F32R, name=f"xT0_sb{f}") for f in range(2)]
            for f in range(2):
                nc.sync.dma_start(out=xT_cur[f][:, :], in_=xT0[f * 128:(f + 1) * 128, :])

            # warmup collective: absorbs the ~75us ncfw cold start under the
            # L0 edge phase. Reduces garbage (the value is never read) so it
            # has no data dependencies and triggers immediately; the in-order
            # cc stream serializes it before the L0 stats AllReduce. Nothing
            # waits on its completion.
            warm_in = dp.tile([128, 1], F32, name="warm_in")
            warm_out = dp.tile([128, 1], F32, addr_space="Shared", name="warm_out")
            nc.gpsimd.collective_compute(
                "AllReduce", ALU.add,
                replica_groups=[list(range(NCORE))],
                ins=[warm_in[:, :]], outs=[warm_out[:, :]])

            xsrc = x0  # gather source (DRAM AP-able handle)

            qctr = [0]

            for l in range(L):
                # ================= edge phase =================
                gats = [None] * NWIN

                def ensure_gather(w, l=l, gats=gats):
                    if gats[w] is not None:
                        return gats[w]
                    g = gp.tile([128, nch, D], FP8, tag="g", name=f"g{l}_{w}")
                    # single-packet coalescing caps at 64 descs/engine (1024
                    # idxs per call); split the window into pieces striped
                    # over the 4 SWDGE queues so descgen runs 4-wide.
                    for off in range(0, cap, 1024):
                        sz = min(1024, cap - off)
                        co = w * (cap // 16) + off // 16
                        nc.gpsimd.dma_gather(
                            out_ap=g[:, off // 128:(off + sz) // 128, :],
                            in_ap=xsrc[:, :],
                            idxs_ap=idx_all[:, co:co + sz // 16],
                            num_idxs=sz, num_idxs_reg=sz,
                            elem_size=D, single_packet=True,
                            queue_num=qctr[0] % 4)
                        qctr[0] += 1
                    gats[w] = g
                    return g

                # Fused edge + dense pipeline: windows stream through; after
                # every 4th window the corresponding 512-node dense block,
                # its stat partials, and its node-major transposes fire.
                preout = [xp.tile([128, NPC], F32, tag=f"pre{f}", name=f"pre{l}_{f}")
                          for f in range(2)]
                redp = cp.tile([128, 16], F32, tag="redp", bufs=2, name=f"redp{l}")
                sqscr = xp.tile([128, 512], F32, tag="sqscr", name=f"sqscr{l}")
                msxTn = [[None] * (NPC // 512) for _ in range(2)]
                for w in range(NWIN):
                    g = ensure_gather(w)
                    for pf in range(1, 4):  # prefetch 3 windows ahead
                        ensure_gather(min(w + pf, NWIN - 1))
                    ps = psE.tile([128, D], F32, tag="pse", name=f"pse{l}_{w}")
                    for c in range(nch // 2):
                        nc.tensor.matmul(ps[:, :], sel[:, w, c, :, :], g[:, 2 * c:2 * c + 2, :],
                                         start=(c == 0), stop=(c == nch // 2 - 1),
                                         perf_mode=DROW)
                    # mean on evacuation (scalar engine; recip is per-partition)
                    msxw = xp.tile([128, D], F32, tag="msx", bufs=4,
                                   name=f"msx{l}_{w}")
                    nc.scalar.activation(msxw[:, :], ps[:, :], ACTF.Copy,
                                         bias=0.0, scale=recip[:, w:w + 1])
                    nb, wi = w // 4, w % 4
                    for f in range(2):
                        if wi == 0:
                            msxTn[f][nb] = xp.tile([128, 512], F32R, tag=f"msxT{f}",
                                                   bufs=2, name=f"msxT{l}_{f}_{nb}")
                        pt = psT.tile([128, 128], F32, tag="pst", name=f"ptm{l}_{w}_{f}")
                        nc.tensor.transpose(pt[:, :], msxw[:, f * 128:(f + 1) * 128],
                                            ident[:, :])
                        nc.vector.tensor_copy(msxTn[f][nb][:, wi * 128:(wi + 1) * 128],
                                              pt[:, :])
                    if wi != 3:
                        continue
                    # dense block for this group of 4 windows
                    cols = slice(nb * 512, (nb + 1) * 512)
                    for f in range(2):
                        pd = psD.tile([128, 512], F32, tag="psd", name=f"pd{l}_{f}_{nb}")
                        fo = slice(f * 128, (f + 1) * 128)
                        nc.tensor.matmul(pd[:, :], wm_sb[l][:, 0, fo], msxTn[0][nb][:, :],
                                         start=True, stop=False)
                        nc.tensor.matmul(pd[:, :], wm_sb[l][:, 1, fo], msxTn[1][nb][:, :],
                                         start=False, stop=False)
                        nc.tensor.matmul(pd[:, :], ws_sb[l][:, 0, fo], xT_cur[0][:, cols],
                                         start=False, stop=False)
                        nc.tensor.matmul(pd[:, :], ws_sb[l][:, 1, fo], xT_cur[1][:, cols],
                                         start=False, stop=False)
                        nc.tensor.matmul(pd[:, :], ew_sb[l][:, fo], hnt[:, cols],
                                         start=False, stop=True)
                        # evacuate + per-block column sums
                        nc.vector.tensor_scalar(preout[f][:, cols], pd[:, :],
                                                1.0, 0.0, ALU.mult, ALU.add,
                                                accum_out=redp[:, f * 4 + nb:f * 4 + nb + 1])
                        # per-block sum of squares on the scalar engine
                        nc.scalar.activation(sqscr[:, :], preout[f][:, cols],
                                             ACTF.Square, bias=0.0, scale=1.0,
                                             accum_out=redp[:, 8 + f * 4 + nb:
                                                            9 + f * 4 + nb])

                # ================= batchnorm stats =================
                red = cp.tile([128, 4], F32, tag="red", bufs=2, name=f"red{l}")
                for f in range(2):
                    nc.vector.tensor_reduce(red[:, f:f + 1], redp[:, f * 4:(f + 1) * 4],
                                            mybir.AxisListType.X, ALU.add)
                    nc.vector.tensor_reduce(red[:, 2 + f:3 + f],
                                            redp[:, 8 + f * 4:8 + (f + 1) * 4],
                                            mybir.AxisListType.X, ALU.add)

                st_in = dp.tile([128, 4], F32, name=f"st_in{l}")
                st_out = dp.tile([128, 4], F32, addr_space="Shared", name=f"st_out{l}")
                nc.scalar.dma_start(out=st_in[:, :], in_=red[:, :])
                nc.gpsimd.collective_compute(
                    "AllReduce", ALU.add,
                    replica_groups=[list(range(NCORE))],
                    ins=[st_in[:, :]], outs=[st_out[:, :]])
                red2 = cp.tile([128, 4], F32, tag="red2", bufs=2, name=f"red2{l}")
                nc.sync.dma_start(out=red2[:, :], in_=st_out[:, :])

                # mu/var -> scale/shift  (all [128, 2], feature-partition form)
                mo = cp.tile([128, 12], F32, tag="mo", bufs=2, name=f"mo{l}")
                mu, ex2, var, vare, sd, rsq = (mo[:, 0:2], mo[:, 2:4], mo[:, 4:6],
                                               mo[:, 6:8], mo[:, 8:10], mo[:, 10:12])
                nc.vector.tensor_scalar(mu, red2[:, 0:2], 1.0 / N, None, ALU.mult)
                nc.vector.tensor_scalar(ex2, red2[:, 2:4], 1.0 / N, None, ALU.mult)
                nc.vector.tensor_tensor(var, mu, mu, ALU.mult)
                nc.vector.tensor_tensor(var, ex2, var, ALU.subtract)
                nc.vector.tensor_scalar(vare, var, EPS, None, ALU.add)
                nc.scalar.activation(sd, vare, ACTF.Sqrt, bias=0.0, scale=1.0)
                nc.vector.reciprocal(rsq, sd)
                gam, bet = gb_sb[l]
                sc = cp.tile([128, 4], F32, tag="sc", bufs=2, name=f"sc{l}")
                scale2, shift2 = sc[:, 0:2], sc[:, 2:4]
                nc.vector.tensor_tensor(scale2, gam[:, :], rsq, ALU.mult)
                nc.vector.tensor_tensor(shift2, mu, scale2, ALU.mult)
                nc.vector.tensor_tensor(shift2, bet[:, :], shift2, ALU.subtract)

                # ===== BN + ReLU feature-major, transpose to node-major, ship
                last = l == L - 1
                xnT = [xp.tile([128, NPC], F32R, tag=f"xnT{f}", name=f"xnT{l}_{f}")
                       for f in range(2)]
                if not last:
                    agi = dp.tile([NPC, D], FP8, name=f"agi{l}")
                    ago = dp.tile([N, D], FP8, addr_space="Shared", name=f"ago{l}")
                xrow = xp.tile([128, NWIN, D], F32 if last else FP8,
                               tag="xrow", name=f"xrow{l}")
                dst = out_ext if last else agi
                dst_ap = dst[:, :].rearrange("(w p) d -> p w d", p=128)
                for nb in range(NWIN // 4):
                    cols = slice(nb * 512, (nb + 1) * 512)
                    for f in range(2):
                        nc.scalar.activation(xnT[f][:, cols], preout[f][:, cols],
                                             ACTF.Relu, bias=shift2[:, f:f + 1],
                                             scale=scale2[:, f:f + 1])
                    for w in range(nb * 4, nb * 4 + 4):
                        for f in range(2):
                            pt = psT.tile([128, 128], F32R, tag="pst",
                                          name=f"ptx{l}_{w}_{f}")
                            nc.tensor.transpose(pt[:, :],
                                                xnT[f][:, w * 128:(w + 1) * 128],
                                                identr[:, :])
                            nc.vector.tensor_copy(xrow[:, w, f * 128:(f + 1) * 128],
                                                  pt[:, :])
                    ws_ = slice(nb * 4, nb * 4 + 4)
                    nc.sync.dma_start(out=dst_ap[:, ws_, :], in_=xrow[:, ws_, :])

                if not last:
                    nc.gpsimd.collective_compute(
                        "AllGather", ALU.bypass,
                        replica_groups=[list(range(NCORE))],
                        ins=[agi[:, :]], outs=[ago[:, :]])
                    xsrc = ago
                    xT_cur = xnT

    nc.compile()
    nc.m = get_hw_module(nc.m)
    return nc


def _preprocess(data, edge, edge_feature):
    """Host-side index preprocessing: sort edges by src window, dedup each
    window's dst set (each distinct row is gathered once; the multi-hot
    selector routes it to every src slot that references it, with count
    weights), window-pad, build count matrices."""
    import ml_dtypes
    src = np.asarray(edge[0], dtype=np.int64)
    dst = np.asarray(edge[1], dtype=np.int64)
    ef = np.asarray(edge_feature, dtype=np.int64)

    wid_e = src // WIN
    order = np.lexsort((src, dst, wid_e))
    src_s = src[order]
    dst_s = dst[order]

    cnt = np.bincount(src, minlength=N)
    recip = (1.0 / np.maximum(cnt, 1)).astype(np.float32)
    H = np.bincount(src * DS + ef, minlength=N * DS).reshape(N, DS)
    Hn = (H * recip[:, None]).astype(np.float32)

    wid = src_s // WIN
    wcnt = np.bincount(wid, minlength=NWING)
    wstart = np.zeros(NWING + 1, np.int64)
    np.cumsum(wcnt, out=wstart[1:])

    uniqs, invs = [], []
    for g in range(NWING):
        u, inv = np.unique(dst_s[wstart[g]:wstart[g + 1]], return_inverse=True)
        uniqs.append(u)
        invs.append(inv)
    nch = max(int(np.ceil(max(len(u) for u in uniqs) / 128)), 1)
    nch += nch % 2  # DoubleRow consumes chunk pairs
    cap = nch * WIN

    idx_pad = np.zeros((NWING, cap), np.int16)
    selw = np.zeros((NWING, cap, 128), np.float16)
    for g in range(NWING):
        a, b = wstart[g], wstart[g + 1]
        idx_pad[g, :len(uniqs[g])] = uniqs[g].astype(np.int16)
        np.add.at(selw[g], (invs[g], src_s[a:b] - g * WIN), 1.0)

    per_core = []
    for w in range(NCORE):
        gsl = slice(w * NWIN, (w + 1) * NWIN)
        nsl = slice(w * NPC, (w + 1) * NPC)
        flat_idx = idx_pad[gsl].reshape(-1)           # [NWIN*cap]
        idx_tile = np.tile(flat_idx.reshape(-1, 16).T, (8, 1)).astype(np.int16)
        # selector: [p, (w c j)] with p = row slot in chunk, j = src slot
        selc = selw[gsl].reshape(NWIN, nch, 128, 128)  # [w, c, p, j]
        sel_tile = np.ascontiguousarray(
            selc.transpose(2, 0, 1, 3).reshape(128, NWIN * cap)
        ).astype(ml_dtypes.float8_e4m3)
        recip_sw = recip[nsl].reshape(NWIN, 128).T.copy()    # [128, NWIN]
        hnt = np.zeros((128, NPC), np.float32)
        hnt[:DS, :] = Hn[nsl].T
        nz = (cnt[nsl] > 0).astype(np.float32)
        hnt[DS, :] = nz
        hnt[DS + 1, :] = nz
        hnt[DS + 2, :] = 1.0
        hnt = hnt.astype(ml_dtypes.bfloat16)
        xT0 = np.ascontiguousarray(
            data.reshape(N, D)[nsl].T.astype(np.float32))
        per_core.append(dict(idx_in=idx_tile, sel_in=sel_tile,
                             recip_in=recip_sw, hnt_in=hnt, xT0=xT0))
    return nch, per_core


def kernel(data, emb_table, W_msg, b_msg, W_self, b_self, W_edge, b_edge,
           bn_gamma, bn_beta, edge, edge_feature):
    data = np.asarray(data)
    nch, per_core = _preprocess(data, np.asarray(edge), np.asarray(edge_feature))

    if nch not in _CACHE:
        _CACHE[nch] = _build(nch)
    nc = _CACHE[nch]

    import ml_dtypes
    x0 = np.ascontiguousarray(data.reshape(N, D).astype(ml_dtypes.float8_e4m3))
    ident = np.eye(128, dtype=np.float32)
    common = {
        "x0": x0, "ident_in": ident, "identr_in": ident,
        "embT": np.ascontiguousarray(np.asarray(emb_table, np.float32).T),
    }
    for l in range(L):
        common[f"wm{l}"] = np.ascontiguousarray(np.asarray(W_msg[l], np.float32))
        common[f"ws{l}"] = np.ascontiguousarray(np.asarray(W_self[l], np.float32))
        common[f"we{l}"] = np.ascontiguousarray(np.asarray(W_edge[l], np.float32))
        ewc = np.zeros((128 - DD, D), np.float32)
        ewc[0] = np.asarray(b_msg[l], np.float32)
        ewc[1] = np.asarray(b_edge[l], np.float32)
        ewc[2] = np.asarray(b_self[l], np.float32)
        common[f"ewc{l}"] = ewc.astype(ml_dtypes.bfloat16)
        common[f"gam{l}"] = np.asarray(bn_gamma[l], np.float32).reshape(D, 1)
        common[f"bet{l}"] = np.asarray(bn_beta[l], np.float32).reshape(D, 1)

    in_maps = [{**common, **pc} for pc in per_core]
    trace = bool(os.environ.get("GNN_TRN_TRACE"))
    res = bass_utils.run_bass_kernel_spmd(
        nc, in_maps, core_ids=list(range(NCORE)), trace=trace)
    if trace:
        global LAST_RESULT
        LAST_RESULT = res
    out = np.concatenate([res.results[c]["out"] for c in range(NCORE)], axis=0)
    return out.reshape(B, S, D).astype(np.float32)


LAST_RESULT = None
